# revision 20
# baseline (speedup 1.0000x reference)
"""Causal multi-head attention block (B=2, T=2048, C=1024, H=16) on 8 TRN2
NeuronCores.

Sharding: Megatron-style tensor parallel over heads. Core r owns heads
{2r, 2r+1} (output dims [128r, 128r+128) of Wq/Wk/Wv). The final output
projection contracts over all of C, so cores AllGather their local yT
shards (concat on the partition axis == feature axis) into yT_full
[C, B*T], then each core computes a 128-column shard of the output:
outT_shard = Wo[128r:128r+128, :] @ yT_full.

Everything on-device is computed in the "transposed" orientation
(feature-major, token-minor) so the TensorEngine contraction axis always
sits on SBUF partitions and the softmax denominator arrives for free via
a ones-column appended to V:

  qT/kT/vT [128, 4096] = W_shard @ x^T          (x^T passed from host)
  ST tile [128k, 512q] = kT_slice.T @ qT_slice  (contract d=64)
  PT = exp(ST * 1/sqrt(d))                      (no max-subtraction: logits
                                                 are ~N(0,1), |S|max ~ 6)
  causal mask: zero PT where k > q via gpsimd.affine_select
  yT [65, 512] += [v | 1].T @ PT                (row 64 = softmax denom)
  yT_norm = yT[0:64] / broadcast(yT[64])

k-tiles are processed in pairs sharing one 2-bank PSUM tile so each EXP
covers 1024 columns (the ACT engine has a ~352-cycle fixed cost per
instruction). The AllGather is split into 4 chunks (per batch x half) so
gather and output projection overlap the remaining attention compute.

Inputs are bf16 (host-side cast); accumulation is f32 in PSUM.
"""

import numpy as np
import ml_dtypes

import concourse.bacc as bacc
import concourse.mybir as mybir
import concourse.tile as tile
from concourse.bass_utils import run_bass_kernel_spmd
from concourse.masks import make_identity

N_CORES = 8
B, T, C, H = 2, 2048, 1024, 16
D = 64                # head dim
HL = H // N_CORES     # heads per core = 2
DL = HL * D           # local feature dim = 128
TT = B * T            # 4096 tokens total
P = 128
NCH = C // P          # 8 contraction chunks
QCH = 512             # q-chunk (moving free dim)
NQC = T // QCH        # 4 q-chunks per batch entry
NKT = T // P          # 16 k-tiles per batch entry
HCH = T // 2          # AllGather chunk = half batch-entry = 1024 tokens
SCALE = 1.0 / np.sqrt(D)

BF = mybir.dt.bfloat16
F32 = mybir.dt.float32
AF = mybir.ActivationFunctionType


def build_graph():
    nc = bacc.Bacc("TRN2", target_bir_lowering=False, debug=False)

    xT = nc.dram_tensor("xT", [C, TT], BF, kind="ExternalInput")
    # all 4 weight shards pre-packed host-side into SBUF layout
    # [p, w, ci, m]: contiguous rows, loaded as 8 parallel DMAs
    wall = nc.dram_tensor("wall", [P, 4 * NCH * DL], BF, kind="ExternalInput")
    out = nc.dram_tensor("out", [DL, TT], F32, kind="ExternalOutput")

    with tile.TileContext(nc) as tc:
        with (
            tc.tile_pool(name="sb", bufs=1) as sb,
            tc.tile_pool(name="ps", bufs=1, space="PSUM") as ps,
            tc.tile_pool(name="dram", bufs=1, space="DRAM") as dram,
        ):
            # ---- loads ----
            w_sb = sb.tile([P, 4 * NCH * DL], BF, name="w_sb")
            WCOLS = 4 * NCH * DL
            for pc in range(8):
                csl = slice(pc * (WCOLS // 8), (pc + 1) * (WCOLS // 8))
                nc.sync.dma_start(w_sb[:, csl], wall[:, csl])
            w4 = w_sb[:].rearrange("p (w a m) -> p w a m", w=4, a=NCH)
            wq_sb, wk_sb, wv_sb, wo_sb = (w4[:, i] for i in range(4))

            ident = sb.tile([P, P], BF, name="ident")
            make_identity(nc, ident)
            # strictly-lower-triangular -1e9 (k > q): masks causal logits on
            # diagonal blocks, injected into the St PSUM group via
            # matmul(ident, mneg)
            mneg = sb.tile([P, P], BF, name="mneg")
            nc.gpsimd.memset(mneg[:], 0.0)
            nc.gpsimd.affine_select(
                out=mneg[:], in_=mneg[:],
                compare_op=mybir.AluOpType.is_ge,
                fill=-1e9, base=0, channel_multiplier=-1, pattern=[[1, P]],
            )
            # [1, 64] bf16 ones: K=1 outer-product broadcast of the softmax
            # denominator across partitions on the TensorEngine
            ones64 = sb.tile([1, D], BF, name="ones64")
            nc.vector.memset(ones64[:], 1.0)

            wsrc = sb.tile([P, QCH], BF, name="wsrc")
            nc.vector.memset(wsrc[:], 0.5)
            for _ in range(24):
                wdst = ps.tile([P, QCH], F32, tag="st", bufs=3, name="wdst")
                nc.tensor.matmul(wdst[:], ident[:], wsrc[:],
                                 start=True, stop=True)

            qT_sb = sb.tile([P, TT], BF, name="qT_sb")
            kT_sb = sb.tile([P, TT], BF, name="kT_sb")
            vT_sb = sb.tile([P, TT], BF, name="vT_sb")
            # v in natural layout, packed per 128-token tile as
            # [headA(64) | 1 | headB(64) | 1] -> 130 columns
            v_sb = sb.tile([P, TT // P, 2 * (D + 1)], BF, name="v_sb")
            nc.gpsimd.memset(v_sb[:], 1.0)

            CHUNKS = [(c * QCH, QCH) for c in range(8)]
            ag_in = [
                dram.tile([DL, cw], BF, name=f"ag_in{c}")
                for c, (c0, cw) in enumerate(CHUNKS)
            ]
            ytf = [
                dram.tile([C, cw], BF, name=f"ytf{c}", addr_space="Shared")
                for c, (c0, cw) in enumerate(CHUNKS)
            ]
            # (b, jq) -> (chunk, col offset within chunk)
            CHUNK_OF = {(b, jq): (b * NQC + jq, 0)
                        for b in range(B) for jq in range(NQC)}

            with tc.tile_pool(name="xp", bufs=1) as xp:
                xT_sb = xp.tile([P, NCH, TT], BF, name="xT_sb")
                # first 512 columns per chunk land fast (small DMAs), the
                # rest streams as one big DMA per chunk; issued from the
                # scalar queue so the sync queue isn't serialized at start
                for ci in range(NCH):
                    nc.scalar.dma_start(
                        xT_sb[:, ci, 0:QCH], xT[ci * P:(ci + 1) * P, 0:QCH]
                    )
                for s0, s1 in ((QCH, 4 * QCH), (4 * QCH, TT)):
                    for ci in range(NCH):
                        nc.scalar.dma_start(
                            xT_sb[:, ci, s0:s1], xT[ci * P:(ci + 1) * P, s0:s1]
                        )

                def proj_group(tch, wsb, dst):
                    tsl = slice(tch * QCH, (tch + 1) * QCH)
                    pj = ps.tile([P, QCH], F32, tag="st", bufs=3,
                                 name="pj")
                    for ci in range(NCH):
                        nc.tensor.matmul(
                            pj[:], wsb[:, ci, :], xT_sb[:, ci, tsl],
                            start=(ci == 0), stop=(ci == NCH - 1),
                        )
                    nc.vector.tensor_copy(dst[:, tsl], pj[:])

                def vtrans(t32):
                    tr = ps.tile([P, P], BF, tag="st", bufs=3, name="tr")
                    nc.tensor.transpose(
                        tr[:], vT_sb[:, t32 * P:(t32 + 1) * P], ident[:]
                    )
                    out_ap = v_sb[:, t32, :].rearrange(
                        "p (h x) -> p h x", h=HL
                    )[:, :, 0:D]
                    in_ap = tr[:].rearrange("p (h x) -> p h x", h=HL)
                    nc.vector.tensor_copy(out_ap, in_ap)

                def attn_compute(b, jq, h):
                    rsl = slice(h * D, (h + 1) * D)
                    q0 = b * T + jq * QCH
                    yt = ps.tile([D + 1, QCH], F32, tag="yt", bufs=2,
                                 name="yt")
                    nkt = 4 * jq + 4
                    for pr in range(nkt // 2):
                        st = ps.tile([P, 2 * QCH], F32, tag="st", bufs=3,
                                     name="st")
                        pt = sb.tile([P, 2 * QCH], BF, tag="pt", bufs=4,
                                     name="pt")
                        for half in range(2):
                            kt = 2 * pr + half
                            k0 = b * T + kt * P
                            i = kt - 4 * jq
                            # diagonal tiles: only q >= kt*128 live; leading
                            # 128 live columns get the -1e9 triangle
                            qv = max(i, 0) * P
                            ssl = slice(half * QCH + qv, (half + 1) * QCH)
                            nc.tensor.matmul(
                                st[:, ssl],
                                kT_sb[rsl, k0:k0 + P],
                                qT_sb[rsl, q0 + qv:q0 + QCH],
                                start=True, stop=(i < 0),
                            )
                            if i >= 0:
                                nc.tensor.matmul(
                                    st[:, half * QCH + qv:
                                       half * QCH + qv + P],
                                    ident[:], mneg[:],
                                    start=False, stop=True,
                                )
                        nc.scalar.activation(
                            pt[:], st[:], AF.Exp, scale=float(SCALE)
                        )
                        for half in range(2):
                            kt = 2 * pr + half
                            qv = max(kt - 4 * jq, 0) * P
                            nc.tensor.matmul(
                                yt[:, qv:QCH],
                                v_sb[:, b * NKT + kt,
                                     h * (D + 1):(h + 1) * (D + 1)],
                                pt[:, half * QCH + qv:(half + 1) * QCH],
                                start=(kt == 0), stop=(kt == nkt - 1),
                            )
                    # denominator row -> SBUF bf16 right away; the rest of
                    # the eviction runs after the next filler block so the
                    # PE queue never waits on it
                    den = sb.tile([1, QCH], BF, tag="den", bufs=4, name="den")
                    nc.vector.tensor_copy(den[:], yt[D:D + 1, :])
                    return yt, den

                def attn_evict(b, jq, h, yt, den):
                    rsl = slice(h * D, (h + 1) * D)
                    bc = ps.tile([D, QCH], F32, tag="st", bufs=3, name="bc")
                    nc.tensor.matmul(
                        bc[:], ones64[:], den[:], start=True, stop=True
                    )
                    rcp = sb.tile([D, QCH], F32, tag="rcp", bufs=3, name="rcp")
                    scr = sb.tile([D, QCH], F32, tag="scr", bufs=3, name="scr")
                    nc.vector.reciprocal_approx_accurate(
                        rcp[:], bc[:], scratch=scr[:]
                    )
                    yn = sb.tile([D, QCH], BF, tag="yn", bufs=4, name="yn")
                    nc.vector.tensor_mul(yn[:], yt[0:D, :], rcp[:])
                    # stream this piece straight into the gather input;
                    # the final pieces are split for latency
                    c, off = CHUNK_OF[(b, jq)]
                    nsp = 2 if (b, jq) == (1, 3) else 1  # final pieces split for latency
                    w = QCH // nsp
                    for s in range(nsp):
                        nc.gpsimd.dma_start(
                            ag_in[c][h * D:(h + 1) * D,
                                     off + s * w:off + (s + 1) * w],
                            yn[:, s * w:(s + 1) * w],
                        )

                def ag_fire(c):
                    nc.gpsimd.collective_compute(
                        "AllGather",
                        mybir.AluOpType.bypass,
                        replica_groups=[list(range(N_CORES))],
                        ins=[ag_in[c][:]],
                        outs=[ytf[c][:]],
                    )

                yf_tiles = {}

                def yf_load(c):
                    c0, cw = CHUNKS[c]
                    yf = sb.tile([P, NCH, QCH], BF, tag="yf", bufs=2,
                                 name="yf")
                    yf_tiles[c] = yf
                    for ci in range(NCH):
                        nc.sync.dma_start(
                            yf[:, ci, :], ytf[c][ci * P:(ci + 1) * P, :]
                        )

                def po_group(c, last=False):
                    c0, cw = CHUNKS[c]
                    yf = yf_tiles[c]
                    po = ps.tile([P, QCH], F32, tag="st", bufs=3, name="po")
                    for ci in range(NCH):
                        nc.tensor.matmul(
                            po[:], wo_sb[:, ci, :],
                            yf[:, ci, :],
                            start=(ci == 0), stop=(ci == NCH - 1),
                        )
                    ob = sb.tile([P, QCH], F32, tag="ob", bufs=2, name="ob")
                    nc.vector.tensor_copy(ob[:], po[:])
                    nsp = 4 if last else 1
                    w = QCH // nsp
                    for s in range(nsp):
                        o0 = c0 + s * w
                        nc.gpsimd.dma_start(
                            out[:, o0:o0 + w], ob[:, s * w:(s + 1) * w]
                        )

                # ---- prologue: b0 projections ----
                for tch in range(4):
                    for wsb, dst in ((wq_sb, qT_sb), (wk_sb, kT_sb),
                                     (wv_sb, vT_sb)):
                        proj_group(tch, wsb, dst)
                    for t32 in range(tch * 4, tch * 4 + 4):
                        vtrans(t32)

                # filler: b1 projections, fed into b0's attention stream to
                # keep the PE dense (HAM warm) while exp gates the AV chain
                filler = []
                for tch in range(4, 8):
                    for wsb, dst in ((wq_sb, qT_sb), (wk_sb, kT_sb),
                                     (wv_sb, vT_sb)):
                        filler.append((proj_group, (tch, wsb, dst)))
                    for t32 in range(tch * 4, tch * 4 + 4):
                        filler.append((vtrans, (t32,)))

                def pop_filler(n):
                    for _ in range(min(n, len(filler))):
                        fn, args = filler.pop(0)
                        fn(*args)

                # ---- b0 attention ----
                for jq in range(NQC):
                    for h in range(HL):
                        yt, den = attn_compute(0, jq, h)
                        pop_filler(jq + 1)
                        if (jq, h) == (3, 0):
                            yf_load(0); po_group(0)
                        attn_evict(0, jq, h, yt, den)
                    pop_filler(2)
                    ag_fire(jq)
                pop_filler(99)

                # ---- b1 attention with O-proj filler ----
                for jq in range(NQC):
                    for h in range(HL):
                        yt, den = attn_compute(1, jq, h)
                        step = (jq, h)
                        if step == (0, 0):
                            yf_load(1); po_group(1)
                        elif step == (1, 0):
                            yf_load(2); po_group(2)
                        elif step == (2, 0):
                            yf_load(3); po_group(3)
                        elif step == (2, 1):
                            yf_load(4); po_group(4)
                        elif step == (3, 0):
                            yf_load(5); po_group(5)
                        attn_evict(1, jq, h, yt, den)
                    ag_fire(NQC + jq)
                yf_load(6)
                po_group(6)
                yf_load(7)
                po_group(7, last=True)

    nc.finalize()
    return nc


_GRAPH = None


def _get_graph():
    global _GRAPH
    if _GRAPH is None:
        _GRAPH = build_graph()
    return _GRAPH


def prepare_in_maps(x, Wq, Wk, Wv, Wo):
    x = np.asarray(x, np.float32)
    Wq = np.asarray(Wq, np.float32)
    Wk = np.asarray(Wk, np.float32)
    Wv = np.asarray(Wv, np.float32)
    Wo = np.asarray(Wo, np.float32)

    bf = ml_dtypes.bfloat16
    xTh = np.ascontiguousarray(x.reshape(TT, C).T).astype(bf)
    in_maps = []
    for r in range(N_CORES):
        sl = slice(r * DL, (r + 1) * DL)
        # pack the 4 transposed weight shards into the SBUF layout
        # [p, w, ci, m] where the shard row index is c = ci*128 + p
        wall = np.empty((P, 4, NCH, DL), np.float32)
        for w, W in enumerate((Wq, Wk, Wv, Wo)):
            wall[:, w] = W[sl].T.reshape(NCH, P, DL).transpose(1, 0, 2)
        in_maps.append({
            "xT": xTh,
            "wall": np.ascontiguousarray(
                wall.reshape(P, 4 * NCH * DL)).astype(bf),
        })
    return in_maps


def assemble_output(results):
    outT = np.concatenate(
        [np.asarray(results[r]["out"], np.float32) for r in range(N_CORES)],
        axis=0,
    )  # [C, TT]
    return np.ascontiguousarray(outT.T).reshape(B, T, C)


def kernel(x, Wq, Wk, Wv, Wo):
    nc = _get_graph()
    in_maps = prepare_in_maps(x, Wq, Wk, Wv, Wo)
    res = run_bass_kernel_spmd(nc, in_maps, core_ids=list(range(N_CORES)))
    return assemble_output(res.results)
